# revision 1
# baseline (speedup 1.0000x reference)
"""Trainium2 Bass kernel for nn_DKT (GAT chain-graph + LSTM network).

Strategy: data-parallel over batch (8 sequences per core x 8 cores).
Per core, tokens live on a padded grid t = s*512 + n (n < 499 real).
All activations are feature-major ([feature-partition, token-free]) so every
dense layer is a PE matmul with bf16 operands and fp32 PSUM accumulation.

Per core:
  - embedding lookups via custom dma_gather (transpose mode): gathers rows of
    bf16 tables, landing feature-major.
  - GAT layers: the graph is a bidirectional chain + self-loops, so message
    passing is a 3-tap stencil along the token axis. Edge scores come from
    folded projections (w_es = W_g1 @ a_src per head); softmax over <=3 taps
    uses taps-on-free-dim layout (no cross-partition ops); alpha is broadcast
    across feature partitions via a DRAM round-trip (SWDGE replicate DMA).
  - LSTM: input-to-hidden precomputed for all timesteps as matmuls
    (r-embedding as a rank-1 outer-product matmul; bias via a ones-row
    matmul). Recurrence keeps gates on partitions, batch 8 on free; sigmoid
    via tanh (host pre-scales i/f/o weights by 0.5) so one ACT op covers all
    gates; cell update uses fused scalar_tensor_tensor ops. States H~ = 2h,
    C~ = 2c absorb 0.5 factors (host folds 0.5 into W_hh and W_out rows).
"""
import sys
sys.path.insert(0, '/opt/trn_rl_repo')

from contextlib import ExitStack

import numpy as np
import ml_dtypes

import concourse.bass as bass
import concourse.bacc as bacc
import concourse.mybir as mybir
import concourse.tile as tile
from concourse import library_config
from concourse.bass_utils import run_bass_kernel_spmd

F32 = mybir.dt.float32
BF16 = mybir.dt.bfloat16
I16 = mybir.dt.int16
AF = mybir.ActivationFunctionType
ALU = mybir.AluOpType
BF = ml_dtypes.bfloat16

B, N, D = 64, 499, 256
NCORES = 8
SEQ = 8            # sequences per core
NP = 512           # padded sequence length
T = SEQ * NP       # tokens per core (4096)
H1 = 8             # GAT1 heads
NEG = -1.0e9


def _wrap_idx(idx_flat):
    """[n] int16 -> [128, n//16] wrap for dma_gather (item i at
    [i % 16, i // 16], replicated to 128 partitions)."""
    w = idx_flat.reshape(-1, 16).T
    return np.tile(w, (8, 1)).copy()


def _grid_idx(arr_core):
    g = np.zeros((SEQ, NP), np.int64)
    g[:, :N] = arr_core
    return g.reshape(-1)


def _attention(nc, attp, dscr, es, ed, nh, uid):
    """Chain 3-tap softmax. es/ed [nh, NP] fp32. Returns DRAM scratch
    [3*nh, NP] bf16 with alpha rows (3*h + tap)."""
    E = attp.tile([nh, 3, NP], F32, tag="E")
    nc.vector.tensor_tensor(E[:, 0, 1:NP], es[:, 0:NP - 1], ed[:, 1:NP],
                            op=ALU.add)
    nc.vector.tensor_tensor(E[:, 1, :], es[:], ed[:], op=ALU.add)
    nc.vector.tensor_tensor(E[:, 2, 0:NP - 1], es[:, 1:NP], ed[:, 0:NP - 1],
                            op=ALU.add)
    nc.vector.memset(E[:, 0, 0:1], NEG)
    nc.vector.memset(E[:, 0, 498:499], NEG)
    nc.vector.memset(E[:, 2, 497:NP], NEG)
    Ew = E[:].rearrange("p a b -> p (a b)")
    nc.vector.scalar_tensor_tensor(Ew, Ew, 0.2, Ew, ALU.mult, ALU.max)
    EX = attp.tile([nh, 3, NP], F32, tag="EX")
    nc.scalar.activation(EX[:].rearrange("p a b -> p (a b)"), Ew, AF.Exp)
    S = attp.tile([nh, NP], F32, tag="S")
    nc.vector.tensor_tensor(S[:], EX[:, 0, :], EX[:, 1, :], op=ALU.add)
    nc.vector.tensor_tensor(S[:], S[:], EX[:, 2, :], op=ALU.add)
    RS = attp.tile([nh, 1, NP], F32, tag="RS")
    nc.vector.reciprocal(RS[:, 0, :], S[:])
    AL = attp.tile([nh, 3, NP], BF16, tag="AL")
    nc.vector.tensor_tensor(AL[:], EX[:], RS[:].to_broadcast([nh, 3, NP]),
                            op=ALU.mult)
    scr = dscr.tile([3 * nh, NP], BF16, tag=f"scr{uid}")
    nc.sync.dma_start(scr[:].rearrange("(h t) f -> h t f", t=3), AL[:])
    return scr


def _repl_alpha(nc, albp, scr, m):
    alb = albp.tile([128, 3, NP], BF16, tag="alb")
    nc.gpsimd.dma_start(
        out=alb[:],
        in_=scr[3 * m:3 * m + 3, :].unsqueeze(0).to_broadcast([128, 3, NP]))
    return alb


def _msg3tap(nc, pool, h, alb, msg_tag):
    """msg[n] = a_self[n]*h[n] + a_left[n]*h[n-1] + a_right[n]*h[n+1]."""
    msg = pool.tile([128, NP], F32, tag=msg_tag)
    tl = pool.tile([128, NP - 1], BF16, tag=msg_tag + "l")
    tr = pool.tile([128, NP - 1], BF16, tag=msg_tag + "r")
    nc.vector.tensor_tensor(msg[:], h[:], alb[:, 1, :], op=ALU.mult)
    nc.vector.tensor_tensor(tl[:], h[:, 0:NP - 1], alb[:, 0, 1:NP],
                            op=ALU.mult)
    nc.vector.tensor_tensor(tr[:], h[:, 1:NP], alb[:, 2, 0:NP - 1],
                            op=ALU.mult)
    nc.vector.tensor_tensor(msg[:, 1:NP], msg[:, 1:NP], tl[:], op=ALU.add)
    nc.vector.tensor_tensor(msg[:, 0:NP - 1], msg[:, 0:NP - 1], tr[:],
                            op=ALU.add)
    return msg


def build_nc(n_steps=N, n_seq=SEQ):
    nc = bacc.Bacc("TRN2", target_bir_lowering=False, debug=False,
                   num_devices=NCORES)

    # ---------------- DRAM inputs ----------------
    d_tbl_p = nc.dram_tensor("tbl_p", [10001, D], BF16, kind="ExternalInput")
    d_tbl_q = nc.dram_tensor("tbl_q", [2001, D], BF16, kind="ExternalInput")
    d_tbl_a = nc.dram_tensor("tbl_a", [11, D], BF16, kind="ExternalInput")
    d_idx = {}
    for nm in ("p", "q", "aff"):   # per-seq wrapped [128, SEQ*32]
        d_idx[nm] = nc.dram_tensor(f"idx_{nm}", [128, SEQ * (NP // 16)], I16,
                                   kind="ExternalInput")
    for nm in ("qn", "pn"):        # whole-grid wrapped [128, T//16]
        d_idx[nm] = nc.dram_tensor(f"idx_{nm}", [128, T // 16], I16,
                                   kind="ExternalInput")
    d_rrow = nc.dram_tensor("r_row", [1, T], BF16, kind="ExternalInput")
    d_wa1 = nc.dram_tensor("wa1", [D, D], BF16, kind="ExternalInput")
    d_wg1 = nc.dram_tensor("wg1", [D, 1024], BF16, kind="ExternalInput")
    d_wes1 = nc.dram_tensor("wes1", [D, H1], BF16, kind="ExternalInput")
    d_wed1 = nc.dram_tensor("wed1", [D, H1], BF16, kind="ExternalInput")
    d_wg2 = nc.dram_tensor("wg2", [1024, D], BF16, kind="ExternalInput")
    d_a2 = nc.dram_tensor("a2", [D, 2], BF16, kind="ExternalInput")
    d_w1s = nc.dram_tensor("w1s", [D, 1024], BF16, kind="ExternalInput")
    d_w2s = nc.dram_tensor("w2s", [D, 1024], BF16, kind="ExternalInput")
    d_w4s = nc.dram_tensor("w4s", [D, 1024], BF16, kind="ExternalInput")
    d_whh = nc.dram_tensor("whh", [D, 1024], BF16, kind="ExternalInput")
    d_brow = nc.dram_tensor("bias_row", [1, 1024], BF16, kind="ExternalInput")
    d_rdir = nc.dram_tensor("rdir_row", [1, 1024], BF16, kind="ExternalInput")
    d_bg1 = nc.dram_tensor("bg1", [128, 8], F32, kind="ExternalInput")
    d_bg2 = nc.dram_tensor("bg2", [128, 2], F32, kind="ExternalInput")
    d_wout = nc.dram_tensor("wout3", [D, 3], BF16, kind="ExternalInput")
    d_bout = nc.dram_tensor("bout", [1, 1], F32, kind="ExternalInput")
    d_y = nc.dram_tensor("y", [SEQ, N], F32, kind="ExternalOutput")

    with tile.TileContext(nc) as tc, ExitStack() as ctx:
        g = ctx.enter_context(tc.tile_pool(name="glob", bufs=1))
        dscr = ctx.enter_context(tc.tile_pool(name="dscr", bufs=1,
                                              space="DRAM"))

        nc.gpsimd.load_library(library_config.mlp)

        def ld(dram, shape, dtype=BF16, tag=None):
            t_ = g.tile(shape, dtype, tag=tag)
            nc.sync.dma_start(t_[:], dram[:])
            return t_

        WA1 = ld(d_wa1[:].rearrange("(a k) m -> k a m", k=128), [128, 2, D], tag="wa1")
        WG1 = ld(d_wg1[:].rearrange("(a k) m -> k a m", k=128), [128, 2, 1024], tag="wg1")
        WES1 = ld(d_wes1[:].rearrange("(a k) m -> k a m", k=128), [128, 2, H1], tag="wes1")
        WED1 = ld(d_wed1[:].rearrange("(a k) m -> k a m", k=128), [128, 2, H1], tag="wed1")
        WG2 = ld(d_wg2[:].rearrange("(a k) m -> k a m", k=128), [128, 8, D], tag="wg2")
        A2 = ld(d_a2[:].rearrange("(a k) m -> k a m", k=128), [128, 2, 2], tag="a2")
        W1S = ld(d_w1s[:].rearrange("(a k) m -> k a m", k=128), [128, 2, 1024], tag="w1s")
        W2S = ld(d_w2s[:].rearrange("(a k) m -> k a m", k=128), [128, 2, 1024], tag="w2s")
        W4S = ld(d_w4s[:].rearrange("(a k) m -> k a m", k=128), [128, 2, 1024], tag="w4s")
        WHH = ld(d_whh[:].rearrange("(a k) m -> k a m", k=128), [128, 2, 1024], tag="whh")
        BROW = ld(d_brow, [1, 1024], tag="brow")
        RDIR = ld(d_rdir, [1, 1024], tag="rdir")
        BG1 = ld(d_bg1, [128, 8], F32, tag="bg1")
        BG2 = ld(d_bg2, [128, 2], F32, tag="bg2")
        WOUT = ld(d_wout[:].rearrange("(a k) m -> k a m", k=128), [128, 2, 3], tag="wout")
        BOUT = ld(d_bout, [1, 1], F32, tag="bout")
        IDX = {nm: ld(d_idx[nm], [128, d_idx[nm].shape[1]], I16, tag=f"idx{nm}")
               for nm in ("p", "q", "aff", "qn", "pn")}
        ONES = g.tile([1, NP], BF16)
        nc.vector.memset(ONES[:], 1.0)

        PRE = g.tile([128, 8, SEQ, NP], BF16)   # LSTM input precompute
        HS = g.tile([128, 2, SEQ, NP], BF16)    # H~ history
        nc.vector.memset(HS[:], 0.0)

        # ============ per-sequence pre-LSTM pipeline ============
        with tc.tile_pool(name="sq2", bufs=2) as sq2, \
             tc.tile_pool(name="sq1", bufs=1) as sq1, \
             tc.tile_pool(name="albp", bufs=2) as albp, \
             tc.tile_pool(name="attp", bufs=1) as attp, \
             tc.tile_pool(name="ps", bufs=4, space="PSUM") as ps, \
             tc.tile_pool(name="pss", bufs=1, space="PSUM") as pss:

            for s in range(n_seq):
                c0 = s * NP
                i0 = s * (NP // 16)

                # --- gathers for this sequence ---
                PT = sq2.tile([128, 2, NP], BF16, tag="PT")
                nc.gpsimd.dma_gather(PT[:], d_tbl_p[:],
                                     IDX["p"][:, i0:i0 + NP // 16],
                                     NP, NP, D, transpose=True)
                QT = sq2.tile([128, 2, NP], BF16, tag="QT")
                nc.gpsimd.dma_gather(QT[:], d_tbl_q[:],
                                     IDX["q"][:, i0:i0 + NP // 16],
                                     NP, NP, D, transpose=True)
                AT = sq1.tile([128, 2, NP], BF16, tag="AT")
                nc.gpsimd.dma_gather(AT[:], d_tbl_a[:],
                                     IDX["aff"][:, i0:i0 + NP // 16],
                                     NP, NP, D, transpose=True)
                RG = sq2.tile([1, NP], BF16, tag="RG")
                nc.sync.dma_start(RG[:], d_rrow[:, c0:c0 + NP])

                # --- affcat: X = p_emb @ Wa1 + Aproj[aff] ---
                XT = sq1.tile([128, 2, NP], BF16, tag="XT")
                for m in range(2):
                    pm = ps.tile([128, NP], F32)
                    for k in range(2):
                        nc.tensor.matmul(pm[:],
                                         WA1[:, k, m * 128:(m + 1) * 128],
                                         PT[:, k, :],
                                         start=(k == 0), stop=(k == 1))
                    nc.vector.tensor_tensor(XT[:, m, :], pm[:], AT[:, m, :],
                                            op=ALU.add)

                # --- GAT1 ---
                h1 = sq1.tile([128, H1, NP], BF16, tag="h1")
                for m in range(H1):
                    pm = ps.tile([128, NP], F32)
                    for k in range(2):
                        nc.tensor.matmul(pm[:],
                                         WG1[:, k, m * 128:(m + 1) * 128],
                                         XT[:, k, :],
                                         start=(k == 0), stop=(k == 1))
                    nc.vector.tensor_copy(h1[:, m, :], pm[:])
                pes = pss.tile([H1, NP], F32, tag="pes")
                for k in range(2):
                    nc.tensor.matmul(pes[:], WES1[:, k, :], XT[:, k, :],
                                     start=(k == 0), stop=(k == 1))
                ped = pss.tile([H1, NP], F32, tag="ped")
                for k in range(2):
                    nc.tensor.matmul(ped[:], WED1[:, k, :], XT[:, k, :],
                                     start=(k == 0), stop=(k == 1))
                es = attp.tile([H1, NP], F32, tag="es")
                ed = attp.tile([H1, NP], F32, tag="ed")
                nc.vector.tensor_copy(es[:], pes[:])
                nc.vector.tensor_copy(ed[:], ped[:])

                scr1 = _attention(nc, attp, dscr, es, ed, H1, f"a{s}")

                # --- messages + ELU -> x1 ---
                x1 = sq1.tile([128, H1, NP], BF16, tag="x1")
                for m in range(H1):
                    alb = _repl_alpha(nc, albp, scr1, m)
                    msg = _msg3tap(nc, sq1, h1[:, m, :], alb, "ms")
                    rz = sq2.tile([128, NP], BF16, tag="rz")
                    mz = sq2.tile([128, NP], BF16, tag="mz")
                    nc.vector.tensor_scalar(rz[:], msg[:], BG1[:, m:m + 1],
                                            0.0, ALU.add, ALU.max)
                    nc.vector.tensor_scalar(mz[:], msg[:], BG1[:, m:m + 1],
                                            0.0, ALU.add, ALU.min)
                    et = sq2.tile([128, NP], BF16, tag="et")
                    nc.scalar.activation(et[:], mz[:], AF.Exp)
                    nc.vector.scalar_tensor_tensor(x1[:, m, :], rz[:], -1.0,
                                                   et[:], ALU.add, ALU.add)

                # --- GAT2 ---
                h2 = sq1.tile([128, 2, NP], BF16, tag="h2")
                for m in range(2):
                    pm = ps.tile([128, NP], F32)
                    for k in range(8):
                        nc.tensor.matmul(pm[:],
                                         WG2[:, k, m * 128:(m + 1) * 128],
                                         x1[:, k, :],
                                         start=(k == 0), stop=(k == 7))
                    nc.vector.tensor_copy(h2[:, m, :], pm[:])
                pes2 = pss.tile([1, NP], F32, tag="pes2")
                for k in range(2):
                    nc.tensor.matmul(pes2[:], A2[:, k, 0:1], h2[:, k, :],
                                     start=(k == 0), stop=(k == 1))
                ped2 = pss.tile([1, NP], F32, tag="ped2")
                for k in range(2):
                    nc.tensor.matmul(ped2[:], A2[:, k, 1:2], h2[:, k, :],
                                     start=(k == 0), stop=(k == 1))
                es2 = attp.tile([1, NP], F32, tag="es")
                ed2 = attp.tile([1, NP], F32, tag="ed")
                nc.vector.tensor_copy(es2[:], pes2[:])
                nc.vector.tensor_copy(ed2[:], ped2[:])

                scr2 = _attention(nc, attp, dscr, es2, ed2, 1, f"b{s}")
                alb2 = _repl_alpha(nc, albp, scr2, 0)

                X2T = sq1.tile([128, 2, NP], BF16, tag="X2T")
                for m in range(2):
                    msg = _msg3tap(nc, sq1, h2[:, m, :], alb2, "m2")
                    nc.vector.tensor_scalar(X2T[:, m, :], msg[:],
                                            BG2[:, m:m + 1], None, ALU.add)

                # --- LSTM input precompute ---
                for m in range(8):
                    pm = ps.tile([128, NP], F32)
                    mm = 0
                    for W_, src in ((W1S, PT), (W2S, QT), (W4S, X2T)):
                        for k in range(2):
                            nc.tensor.matmul(
                                pm[:], W_[:, k, m * 128:(m + 1) * 128],
                                src[:, k, :], start=(mm == 0), stop=False)
                            mm += 1
                    nc.tensor.matmul(pm[:], BROW[:, m * 128:(m + 1) * 128],
                                     ONES[:], start=False, stop=False)
                    nc.tensor.matmul(pm[:], RDIR[:, m * 128:(m + 1) * 128],
                                     RG[:], start=False, stop=True)
                    nc.vector.tensor_copy(PRE[:, m, s, :], pm[:])

        # ============ LSTM recurrence ============
        with tc.tile_pool(name="lstm", bufs=3) as lp, \
             tc.tile_pool(name="lps", bufs=2, space="PSUM") as lps:
            CST = None
            for n in range(n_steps):
                gt = lp.tile([128, 64], F32, tag="gt")
                if n == 0:
                    nc.vector.tensor_copy(gt[:], PRE[:, :, :, 0])
                else:
                    pg = lps.tile([128, 64], F32)
                    for j in range(8):
                        for kk in range(2):
                            nc.tensor.matmul(
                                pg[:, j * 8:(j + 1) * 8],
                                WHH[:, kk, j * 128:(j + 1) * 128],
                                HS[:, kk, :, n - 1],
                                start=(kk == 0), stop=(kk == 1))
                    nc.vector.tensor_tensor(gt[:], pg[:], PRE[:, :, :, n],
                                            op=ALU.add)
                tt = lp.tile([128, 64], F32, tag="tt")
                nc.scalar.activation(tt[:], gt[:], AF.Tanh)
                cn = lp.tile([128, 16], F32, tag="c")
                if n == 0:
                    nc.vector.scalar_tensor_tensor(
                        cn[:], tt[:, 0:16], 1.0, tt[:, 32:48],
                        ALU.add, ALU.mult)
                else:
                    av = lp.tile([128, 16], F32, tag="av")
                    bv = lp.tile([128, 16], F32, tag="bv")
                    nc.vector.scalar_tensor_tensor(
                        av[:], tt[:, 16:32], 1.0, CST[:], ALU.add, ALU.mult)
                    nc.vector.scalar_tensor_tensor(
                        bv[:], tt[:, 0:16], 1.0, tt[:, 32:48],
                        ALU.add, ALU.mult)
                    nc.vector.scalar_tensor_tensor(
                        cn[:], av[:], 0.5, bv[:], ALU.mult, ALU.add)
                CST = cn
                tcn = lp.tile([128, 16], F32, tag="tc")
                nc.scalar.activation(tcn[:], cn[:], AF.Tanh, scale=0.5)
                nc.vector.scalar_tensor_tensor(
                    HS[:, :, :, n], tt[:, 48:64], 1.0, tcn[:],
                    ALU.add, ALU.mult)

        # ============ output ============
        with tc.tile_pool(name="outp", bufs=1) as op_, \
             tc.tile_pool(name="ops", bufs=2, space="PSUM") as ops_:
            QNT = op_.tile([128, SEQ, 2, NP], BF16, tag="qnt")
            PNT = op_.tile([128, SEQ, 2, NP], BF16, tag="pnt")
            for s in range(n_seq):
                i0 = s * (NP // 16)
                nc.gpsimd.dma_gather(QNT[:, s, :, :], d_tbl_q[:],
                                     IDX["qn"][:, i0:i0 + NP // 16],
                                     NP, NP, D, transpose=True)
                nc.gpsimd.dma_gather(PNT[:, s, :, :], d_tbl_p[:],
                                     IDX["pn"][:, i0:i0 + NP // 16],
                                     NP, NP, D, transpose=True)
            for s in range(n_seq):
                c0 = s * NP
                py = ops_.tile([1, NP], F32)
                nc.tensor.matmul(py[:], WOUT[:, 0, 0:1], HS[:, 0, s, :],
                                 start=True, stop=False)
                nc.tensor.matmul(py[:], WOUT[:, 1, 0:1], HS[:, 1, s, :],
                                 start=False, stop=False)
                for kk in range(2):
                    nc.tensor.matmul(py[:], WOUT[:, kk, 1:2],
                                     QNT[:, s, kk, :],
                                     start=False, stop=False)
                for kk in range(2):
                    nc.tensor.matmul(py[:], WOUT[:, kk, 2:3],
                                     PNT[:, s, kk, :],
                                     start=False, stop=(kk == 1))
                ys = op_.tile([1, NP], F32, tag="ys")
                nc.scalar.activation(ys[:], py[:], AF.Sigmoid, bias=BOUT[:])
                nc.sync.dma_start(d_y[s, :], ys[0:1, 0:N])

    nc.compile()
    return nc


def _prep_inputs(inputs):
    f32 = lambda k: np.asarray(inputs[k], np.float32)
    emb_p, emb_q = f32('emb_p'), f32('emb_q')
    emb_r, emb_aff = f32('emb_r'), f32('emb_aff')
    W_affcat, b_affcat = f32('W_affcat'), f32('b_affcat')
    W_g1, a_src1, a_dst1, b_g1 = (f32('W_g1'), f32('a_src1'), f32('a_dst1'),
                                  f32('b_g1'))
    W_g2, a_src2, a_dst2, b_g2 = (f32('W_g2'), f32('a_src2'), f32('a_dst2'),
                                  f32('b_g2'))
    W_ih, W_hh, b_ih, b_hh = (f32('W_ih'), f32('W_hh'), f32('b_ih'),
                              f32('b_hh'))
    W_out, b_out = f32('W_out'), f32('b_out')

    Aproj = emb_aff @ W_affcat[D:] + b_affcat
    Wg1r = W_g1.reshape(D, H1, 128)
    w_es1 = np.einsum('dhf,hf->dh', Wg1r, a_src1)
    w_ed1 = np.einsum('dhf,hf->dh', Wg1r, a_dst1)
    a2 = np.stack([a_src2[0], a_dst2[0]], axis=1)

    gs = np.ones((4 * D, 1), np.float32)
    gs[0:D] = 0.5; gs[D:2 * D] = 0.5; gs[3 * D:] = 0.5
    W1s = (W_ih[:, 0:D] * gs).T
    W2s = (W_ih[:, D:2 * D] * gs).T
    W3 = W_ih[:, 2 * D:3 * D]
    W4s = (W_ih[:, 3 * D:4 * D] * gs).T
    bias_comb = (b_ih + b_hh + emb_r[0] @ W3.T) * gs[:, 0]
    r_dir = ((emb_r[1] - emb_r[0]) @ W3.T) * gs[:, 0]
    W_hh_s = (W_hh * gs * 0.5).T
    wout3 = np.stack([W_out[0:D, 0] * 0.5, W_out[D:2 * D, 0],
                      W_out[2 * D:3 * D, 0]], axis=1)

    shared = {
        'tbl_p': emb_p.astype(BF), 'tbl_q': emb_q.astype(BF),
        'tbl_a': Aproj.astype(BF),
        'wa1': W_affcat[:D].astype(BF),
        'wg1': W_g1.astype(BF), 'wes1': w_es1.astype(BF),
        'wed1': w_ed1.astype(BF),
        'wg2': W_g2.astype(BF), 'a2': a2.astype(BF),
        'w1s': W1s.astype(BF), 'w2s': W2s.astype(BF), 'w4s': W4s.astype(BF),
        'whh': W_hh_s.astype(BF),
        'bias_row': bias_comb[None, :].astype(BF),
        'rdir_row': r_dir[None, :].astype(BF),
        'bg1': b_g1.reshape(8, 128).T.copy().astype(np.float32),
        'bg2': b_g2.reshape(2, 128).T.copy().astype(np.float32),
        'wout3': wout3.astype(BF),
        'bout': b_out.reshape(1, 1).astype(np.float32),
    }

    p = np.asarray(inputs['p']); q = np.asarray(inputs['q'])
    r = np.asarray(inputs['r']); aff = np.asarray(inputs['aff'])
    q_next = np.asarray(inputs['q_next']); p_next = np.asarray(inputs['p_next'])

    def per_seq_wrap(arr_core):
        grid = np.zeros((SEQ, NP), np.int64)
        grid[:, :N] = arr_core
        cols = [_wrap_idx(grid[s].astype(np.int16)) for s in range(SEQ)]
        return np.concatenate(cols, axis=1)  # [128, SEQ*32]

    in_maps = []
    for c in range(NCORES):
        sl = slice(c * SEQ, (c + 1) * SEQ)
        m = dict(shared)
        m['idx_p'] = per_seq_wrap(p[sl])
        m['idx_q'] = per_seq_wrap(q[sl])
        m['idx_aff'] = per_seq_wrap(aff[sl])
        m['idx_qn'] = per_seq_wrap(q_next[sl])
        m['idx_pn'] = per_seq_wrap(p_next[sl])
        rg = np.zeros((SEQ, NP), np.float32)
        rg[:, :N] = r[sl]
        m['r_row'] = rg.reshape(1, T).astype(BF)
        in_maps.append(m)
    return in_maps


_NC_CACHE = {}
TRACE = False
LAST_RESULT = None


def kernel(**inputs):
    global LAST_RESULT
    in_maps = _prep_inputs(inputs)
    if 'nc' not in _NC_CACHE:
        _NC_CACHE['nc'] = build_nc()
    nc = _NC_CACHE['nc']
    res = run_bass_kernel_spmd(nc, in_maps, core_ids=list(range(NCORES)),
                               trace=TRACE)
    LAST_RESULT = res
    y = np.concatenate([res.results[c]['y'] for c in range(NCORES)], axis=0)
    return y.reshape(B, N, 1).astype(np.float32)


if __name__ == "__main__":
    data = np.load('/root/problem/work/inputs.npz')
    inp = {k: data[k] for k in data.files}
    y = kernel(**inp)
    exp = np.load('/root/problem/work/expected.npy')
    err = np.abs(y - exp).max()
    print("max abs err:", err, "rel:", err / np.abs(exp).max())

